# revision 1
# baseline (speedup 1.0000x reference)
"""Trainium2 Bass kernel for nn_FDM_46394236731667.

Computes, per batch b (b = 0..7, one NeuronCore each):
    f1,f2,f3 = fm{1,2,3}[b].reshape(C, HW)
    qn  = f1 / max(||f1||_col, eps)  (column-wise L2 over channels)
    s_k = -(qn^T @ (f_k / max(||f_k||_col, eps)))          k in {2,3}
    a_k = softmax(s_k, axis=-1)
    out[b] = f1 + 0.001 * (f2 @ a2^T + f3 @ a3^T)

Implementation notes:
  - Scores are computed TRANSPOSED (tiles [m_partition, n_free]) so that the
    key-norm scale r_k[m] is a per-partition scalar folded into the ACT exp
    instruction (out = exp(scale[p] * psum)), the softmax denominator is a
    partition-reduction done with an all-ones matmul (result arrives already
    broadcast across partitions), and the P@V matmul consumes the exp tiles
    directly with no transposes of probability tiles.
  - Query norms r1[n] ride on the free axis, so they are pre-folded into the
    bf16 operand qn = f1 * broadcast(r1).
  - Matmul dtypes: scores bf16; exp-probs and value matmuls fp8(e4m3).
    The output is fm1 + 0.001*(attention terms), so low-precision attention
    arithmetic perturbs the output by ~1e-6 relative.
  - Softmax skips the max-subtraction: scores are cosine similarities in
    [-1, 1], so exp() cannot overflow.
"""
import os
import sys

for _p in ("/opt/trn_rl_repo", "/root/.axon_site/_ro/trn_rl_repo"):
    if os.path.isdir(_p) and _p not in sys.path:
        sys.path.insert(0, _p)

import numpy as np

import concourse.bass as bass
import concourse.tile as tile
from concourse import bacc, mybir
from concourse.bass_utils import run_bass_kernel_spmd
from concourse.masks import make_identity

B, C, H, W = 8, 512, 56, 56
HW = H * W            # 3136
P = 128
CC = C // P           # 4 channel chunks
NMC = 25              # m chunks: 24 x 128 + 1 x 64
MTAIL = HW - 24 * P   # 64
NNC = 7               # n chunks
NW = HW // NNC        # 448
EPS = 1e-12
FACTOR = 0.001

dt = mybir.dt
F32, BF16, FP8 = dt.float32, dt.bfloat16, dt.float8e4

TRACE = False
_cached_nc = None


def _mw(mc):
    return P if mc < NMC - 1 else MTAIL


def _build_kernel_body(tc, out_ap, fm1, fm2, fm3):
    nc = tc.nc
    sb = tc._probe_sb_pool  # set by caller
    ps = tc._probe_ps_pool

    # ---- constants ----
    ident = sb.tile([P, P], BF16, tag="ident", name="ident")
    make_identity(nc, ident)
    ones128 = sb.tile([P, P], FP8, tag="ones128", name="ones128")
    nc.vector.memset(ones128, 1.0)
    ones_col = sb.tile([P, 1], FP8, tag="ones_col", name="ones_col")
    nc.vector.memset(ones_col, 1.0)

    # =======================================================================
    # f1 phase: qn = f1 * broadcast(1/max(||f1||, eps))
    # =======================================================================
    fsq1 = []
    for cc in range(CC):
        fr = sb.tile([P, HW], F32, tag="fraw", bufs=2, name=f"f1raw_{cc}")
        nc.sync.dma_start(fr, fm1[cc * P:(cc + 1) * P, :])
        t8 = sb.tile([P, HW], FP8, tag="fsq", bufs=4, name=f"fsq1_{cc}")
        nc.scalar.square(t8, fr)
        fsq1.append(t8)

    # rbf[p, n] = 1/max(sqrt(sum_c f1[c,n]^2), eps), broadcast over partitions
    rbf = sb.tile([P, HW], F32, tag="rbf", name="rbf")
    for j in range(NNC):
        js = slice(j * NW, (j + 1) * NW)
        ssb = ps.tile([P, NW], F32, tag="sp", bufs=3, name=f"ss1b_{j}")
        for cc in range(CC):
            nc.tensor.matmul(ssb, ones128, fsq1[cc][:, js],
                             start=(cc == 0), stop=(cc == CC - 1))
        ns = sb.tile([P, NW], F32, tag="rtmp", bufs=2, name=f"ns1_{j}")
        nc.scalar.sqrt(ns, ssb)
        nc.vector.tensor_scalar_max(ns, ns, EPS)
        nc.vector.reciprocal(rbf[:, js], ns)

    qn = []
    for cc in range(CC):
        fr = sb.tile([P, HW], F32, tag="fraw", bufs=2, name=f"f1raw2_{cc}")
        nc.sync.dma_start(fr, fm1[cc * P:(cc + 1) * P, :])
        q = sb.tile([P, HW], BF16, tag="qn", bufs=4, name=f"qn_{cc}")
        nc.vector.tensor_mul(q, fr, rbf)
        qn.append(q)

    # =======================================================================
    # f2/f3 phases: bf16 copy, fp8 transpose, per-column key norms
    # =======================================================================
    def key_phase(fm, label):
        fb, fsq = [], []
        for cc in range(CC):
            fr = sb.tile([P, HW], F32, tag="fraw", bufs=2, name=f"{label}raw_{cc}")
            nc.sync.dma_start(fr, fm[cc * P:(cc + 1) * P, :])
            b16 = sb.tile([P, HW], BF16, tag=f"{label}b", bufs=4,
                          name=f"{label}b_{cc}")
            nc.vector.tensor_copy(b16, fr)
            t8 = sb.tile([P, HW], FP8, tag="fsq", bufs=4, name=f"{label}sq_{cc}")
            nc.scalar.square(t8, fr)
            fb.append(b16)
            fsq.append(t8)

        # ss columns: ssc[:mw, mc] = sum_c f[c, mc*128+p]^2
        ssc = ps.tile([P, NMC], F32, tag="cs", bufs=2, name=f"ssc_{label}")
        for mc in range(NMC):
            mw = _mw(mc)
            msl = slice(mc * P, mc * P + mw)
            for cc in range(CC):
                nc.tensor.matmul(ssc[:mw, mc:mc + 1], fsq[cc][:, msl], ones_col,
                                 start=(cc == 0), stop=(cc == CC - 1))
        nrm = sb.tile([P, NMC], F32, tag="rtmp2", bufs=2, name=f"nrm_{label}")
        nc.scalar.sqrt(nrm, ssc)
        nc.vector.tensor_scalar_max(nrm, nrm, EPS)
        rcp = sb.tile([P, NMC], F32, tag="rtmp3", bufs=2, name=f"rcp_{label}")
        nc.vector.reciprocal(rcp, nrm)
        rneg = sb.tile([P, NMC], F32, tag=f"rneg_{label}", name=f"rneg_{label}")
        nc.vector.tensor_scalar_mul(rneg, rcp, -1.0)

        # fT[mc][p, c] = f[c, mc*128+p]  (fp8), for the value matmuls
        fT = [sb.tile([P, C], FP8, tag=f"{label}T", bufs=NMC,
                      name=f"{label}T_{mc}") for mc in range(NMC)]
        for cc in range(CC):
            for mc in range(NMC):
                mw = _mw(mc)
                msl = slice(mc * P, mc * P + mw)
                tp = ps.tile([P, P], BF16, tag="sp", bufs=3,
                             name=f"tp_{label}_{cc}_{mc}")
                nc.tensor.transpose(tp[:mw, :], fb[cc][:, msl], ident)
                nc.vector.tensor_copy(fT[mc][:mw, cc * P:(cc + 1) * P],
                                      tp[:mw, :])
        return fb, fT, rneg

    f2b, f2T, rneg2 = key_phase(fm2, "k2")
    f3b, f3T, rneg3 = key_phase(fm3, "k3")

    # =======================================================================
    # main loop over n chunks
    # =======================================================================
    for j in range(NNC):
        js = slice(j * NW, (j + 1) * NW)

        f1s = []
        for cc in range(CC):
            fs = sb.tile([P, NW], F32, tag="f1s", bufs=5, name=f"f1s_{j}_{cc}")
            nc.sync.dma_start(fs, fm1[cc * P:(cc + 1) * P, js])
            f1s.append(fs)

        outs = [None] * CC
        for mat, (fb, fT, rneg) in ((2, (f2b, f2T, rneg2)),
                                    (3, (f3b, f3T, rneg3))):
            # scores (transposed) + exp
            E = []
            for mc in range(NMC):
                mw = _mw(mc)
                msl = slice(mc * P, mc * P + mw)
                sp = ps.tile([P, NW], F32, tag="sp", bufs=3,
                             name=f"sp_{j}_{mat}_{mc}")
                for cc in range(CC):
                    nc.tensor.matmul(sp[:mw, :], fb[cc][:, msl], qn[cc][:, js],
                                     start=(cc == 0), stop=(cc == CC - 1))
                e = sb.tile([P, NW], FP8, tag=f"E{mat}", bufs=NMC,
                            name=f"E{mat}_{j}_{mc}")
                nc.scalar.activation(e[:mw, :], sp[:mw, :],
                                     mybir.ActivationFunctionType.Exp,
                                     bias=0.0, scale=rneg[:mw, mc:mc + 1])
                E.append(e)

            # softmax denominator (broadcast over partitions) -> FACTOR/denom
            cs = ps.tile([P, NW], F32, tag="cs", bufs=2, name=f"cs_{j}_{mat}")
            for mc in range(NMC):
                mw = _mw(mc)
                nc.tensor.matmul(cs, ones128[:mw, :], E[mc][:mw, :],
                                 start=(mc == 0), stop=(mc == NMC - 1))
            rs = sb.tile([P, NW], F32, tag="rs", bufs=4, name=f"rs_{j}_{mat}")
            nc.vector.reciprocal(rs, cs)
            nc.vector.tensor_scalar_mul(rs, rs, FACTOR)

            # values: nu[c, n] = sum_m fT[m, c] * E[m, n]
            for cc in range(CC):
                vp = ps.tile([P, NW], F32, tag="vp", bufs=2,
                             name=f"vp_{j}_{mat}_{cc}")
                for mc in range(NMC):
                    mw = _mw(mc)
                    nc.tensor.matmul(vp, fT[mc][:mw, cc * P:(cc + 1) * P],
                                     E[mc][:mw, :],
                                     start=(mc == 0), stop=(mc == NMC - 1))
                tmp = sb.tile([P, NW], F32, tag="t", bufs=2,
                              name=f"t_{j}_{mat}_{cc}")
                nc.vector.tensor_mul(tmp, vp, rs)
                if mat == 2:
                    o = sb.tile([P, NW], F32, tag="outs", bufs=5,
                                name=f"o_{j}_{cc}")
                    nc.vector.tensor_add(o, tmp, f1s[cc])
                    outs[cc] = o
                else:
                    o = outs[cc]
                    nc.vector.tensor_add(o, o, tmp)
                    nc.sync.dma_start(out_ap[cc * P:(cc + 1) * P, js], o)


def _build():
    nc = bacc.Bacc("TRN2", target_bir_lowering=False, debug=False,
                   num_devices=B)
    fm1 = nc.dram_tensor("fm1", [C, HW], F32, kind="ExternalInput").ap()
    fm2 = nc.dram_tensor("fm2", [C, HW], F32, kind="ExternalInput").ap()
    fm3 = nc.dram_tensor("fm3", [C, HW], F32, kind="ExternalInput").ap()
    out = nc.dram_tensor("out", [C, HW], F32, kind="ExternalOutput").ap()

    with tile.TileContext(nc) as tc:
        with tc.tile_pool(name="sb", bufs=1) as sb, \
             tc.tile_pool(name="ps", bufs=1, space="PSUM") as ps:
            tc._probe_sb_pool = sb
            tc._probe_ps_pool = ps
            _build_kernel_body(tc, out, fm1, fm2, fm3)
    nc.compile()
    return nc


def _get_nc():
    global _cached_nc
    if _cached_nc is None:
        _cached_nc = _build()
    return _cached_nc


def kernel(**inputs):
    fm1 = np.ascontiguousarray(
        np.asarray(inputs["fm1"], dtype=np.float32).reshape(B, C, HW))
    fm2 = np.ascontiguousarray(
        np.asarray(inputs["fm2"], dtype=np.float32).reshape(B, C, HW))
    fm3 = np.ascontiguousarray(
        np.asarray(inputs["fm3"], dtype=np.float32).reshape(B, C, HW))

    nc = _get_nc()
    in_maps = [{"fm1": fm1[b], "fm2": fm2[b], "fm3": fm3[b]} for b in range(B)]
    res = run_bass_kernel_spmd(nc, in_maps, core_ids=list(range(B)),
                               trace=TRACE)
    kernel.last_results = res
    out = np.stack([res.results[b]["out"] for b in range(B)])
    return out.reshape(B, C, H, W).astype(np.float32)


if __name__ == "__main__":
    rng = np.random.default_rng(0)
    ins = {k: rng.standard_normal((B, C, H, W)).astype(np.float32)
           for k in ("fm1", "fm2", "fm3")}
    o = kernel(**ins)
    print("out shape", o.shape, o.dtype)


# revision 2
# speedup vs baseline: 4.7802x; 4.7802x over previous
"""Trainium2 Bass kernel for nn_FDM_46394236731667.

Computes, per batch b (b = 0..7, one NeuronCore each):
    f1,f2,f3 = fm{1,2,3}[b].reshape(C, HW)
    qn  = f1 / max(||f1||_col, eps)  (column-wise L2 over channels)
    s_k = -(qn^T @ (f_k / max(||f_k||_col, eps)))          k in {2,3}
    a_k = softmax(s_k, axis=-1)
    out[b] = f1 + 0.001 * (f2 @ a2^T + f3 @ a3^T)

Implementation notes:
  - Scores are computed TRANSPOSED (tiles [m_partition, n_free]) so that the
    key-norm scale r_k[m] is a per-partition scalar folded into the ACT exp
    instruction (out = exp(scale[p] * psum)), the softmax denominator is a
    partition-reduction done with an all-ones matmul (result arrives already
    broadcast across partitions), and the P@V matmul consumes the exp tiles
    directly with no transposes of probability tiles.
  - Query norms r1[n] ride on the free axis, so they are pre-folded into the
    query operand qn = 16 * f1 * broadcast(r1) (the 16 makes fp8 quantization
    of unit-norm entries land in the normal range; the matching 1/16 is folded
    into the exp scale).
  - All three big matmul families (scores, softmax-denominator, values) run in
    fp8(e4m3) with perf_mode=DoubleRow: operands are stored with the
    contraction-chunk index as a middle AP dim so each matmul contracts 256
    rows. The output is fm1 + 0.001*(attention terms), so low-precision
    attention arithmetic perturbs the output by ~1e-6 relative.
  - Softmax skips the max-subtraction: scores are cosine similarities in
    [-1, 1], so exp() cannot overflow.
"""
import os
import sys

for _p in ("/opt/trn_rl_repo", "/root/.axon_site/_ro/trn_rl_repo"):
    if os.path.isdir(_p) and _p not in sys.path:
        sys.path.insert(0, _p)

import numpy as np

import concourse.bass as bass
import concourse.tile as tile
from concourse import bacc, mybir
from concourse.bass_utils import run_bass_kernel_spmd
from concourse.masks import make_identity

B, C, H, W = 8, 512, 56, 56
HW = H * W            # 3136
P = 128
CC = C // P           # 4 channel chunks
NMC = 25              # m chunks: 24 x 128 + 1 x 64
MTAIL = HW - 24 * P   # 64
NNC = 7               # n chunks
NW = HW // NNC        # 448
EPS = 1e-12
FACTOR = 0.001
QSCALE = 16.0         # fp8 headroom scale on qn; 1/QSCALE folded into exp

dt = mybir.dt
F32, BF16, FP8 = dt.float32, dt.bfloat16, dt.float8e4
DR = mybir.MatmulPerfMode.DoubleRow

TRACE = False
_cached_nc = None


def _mw(mc):
    return P if mc < NMC - 1 else MTAIL


def _build_kernel_body(tc, out_ap, fm1, fm2, fm3):
    nc = tc.nc
    sb = tc._probe_sb_pool
    ps = tc._probe_ps_pool

    # ---- constants ----
    ident = sb.tile([P, P], BF16, tag="ident", name="ident")
    make_identity(nc, ident)
    ones128 = sb.tile([P, 2, P], FP8, tag="ones128", name="ones128")
    nc.vector.memset(ones128, 1.0)
    ones_col = sb.tile([P, 1], FP8, tag="ones_col", name="ones_col")
    nc.vector.memset(ones_col, 1.0)

    # =======================================================================
    # f1 phase: qn = QSCALE * f1 * broadcast(1/max(||f1||, eps)), fp8,
    # stored as [P, CC, HW] so score matmuls can take [P, 2, NW] slices.
    # =======================================================================
    fsq1 = []
    for cc in range(CC):
        fr = sb.tile([P, HW], F32, tag="fraw", bufs=2, name=f"f1raw_{cc}")
        nc.sync.dma_start(fr, fm1[cc * P:(cc + 1) * P, :])
        t8 = sb.tile([P, HW], FP8, tag="fsq", bufs=4, name=f"fsq1_{cc}")
        nc.scalar.square(t8, fr)
        fsq1.append(t8)

    rbf = sb.tile([P, HW], F32, tag="rbf", name="rbf")
    for j in range(NNC):
        js = slice(j * NW, (j + 1) * NW)
        ssb = ps.tile([P, NW], F32, tag="sp", bufs=3, name=f"ss1b_{j}")
        for cc in range(CC):
            nc.tensor.matmul(ssb, ones128[:, 0, :], fsq1[cc][:, js],
                             start=(cc == 0), stop=(cc == CC - 1))
        ns = sb.tile([P, NW], F32, tag="rtmp", bufs=2, name=f"ns1_{j}")
        nc.scalar.sqrt(ns, ssb)
        nc.vector.tensor_scalar_max(ns, ns, EPS)
        nc.vector.reciprocal(rbf[:, js], ns)
    # QSCALE headroom for the fp8 cast of qn
    nc.vector.tensor_scalar_mul(rbf, rbf, QSCALE)

    qn = sb.tile([P, CC, HW], FP8, tag="qn", name="qn")
    for cc in range(CC):
        fr = sb.tile([P, HW], F32, tag="fraw", bufs=2, name=f"f1raw2_{cc}")
        nc.sync.dma_start(fr, fm1[cc * P:(cc + 1) * P, :])
        nc.vector.tensor_mul(qn[:, cc, :], fr, rbf)

    # =======================================================================
    # f2/f3 phases: fp8 key operand [P, CC, HW], fp8 transpose [P, NMC, C],
    # per-column key norms (as [P, NMC] columns for the exp scale)
    # =======================================================================
    def key_phase(fm, label):
        fb = sb.tile([P, CC, HW], FP8, tag=f"{label}b", name=f"{label}b")
        fT = sb.tile([P, NMC, C], FP8, tag=f"{label}T", name=f"{label}T")
        fsq = []
        b16s = []
        for cc in range(CC):
            fr = sb.tile([P, HW], F32, tag="fraw", bufs=2, name=f"{label}raw_{cc}")
            nc.sync.dma_start(fr, fm[cc * P:(cc + 1) * P, :])
            nc.vector.tensor_copy(fb[:, cc, :], fr)
            b16 = sb.tile([P, HW], BF16, tag="b16", bufs=2, name=f"{label}b16_{cc}")
            nc.vector.tensor_copy(b16, fr)
            t8 = sb.tile([P, HW], FP8, tag="fsq", bufs=4, name=f"{label}sq_{cc}")
            nc.scalar.square(t8, fr)
            fsq.append(t8)
            b16s.append(b16)
            # transposes for this channel chunk (consume b16 before rotation)
            for mc in range(NMC):
                mw = _mw(mc)
                msl = slice(mc * P, mc * P + mw)
                tp = ps.tile([P, P], BF16, tag="sp", bufs=3,
                             name=f"tp_{label}_{cc}_{mc}")
                nc.tensor.transpose(tp[:mw, :], b16[:, msl], ident)
                nc.vector.tensor_copy(fT[:mw, mc, cc * P:(cc + 1) * P],
                                      tp[:mw, :])

        # ss columns: ssc[:mw, mc] = sum_c f[c, mc*128+p]^2
        ssc = ps.tile([P, NMC], F32, tag="cs", bufs=2, name=f"ssc_{label}")
        for mc in range(NMC):
            mw = _mw(mc)
            msl = slice(mc * P, mc * P + mw)
            for cc in range(CC):
                nc.tensor.matmul(ssc[:mw, mc:mc + 1], fsq[cc][:, msl], ones_col,
                                 start=(cc == 0), stop=(cc == CC - 1))
        nrm = sb.tile([P, NMC], F32, tag="rtmp2", bufs=2, name=f"nrm_{label}")
        nc.scalar.sqrt(nrm, ssc)
        nc.vector.tensor_scalar_max(nrm, nrm, EPS)
        rcp = sb.tile([P, NMC], F32, tag="rtmp3", bufs=2, name=f"rcp_{label}")
        nc.vector.reciprocal(rcp, nrm)
        rneg = sb.tile([P, NMC], F32, tag=f"rneg_{label}", name=f"rneg_{label}")
        nc.vector.tensor_scalar_mul(rneg, rcp, -1.0 / QSCALE)
        return fb, fT, rneg

    f2b, f2T, rneg2 = key_phase(fm2, "k2")
    f3b, f3T, rneg3 = key_phase(fm3, "k3")

    # =======================================================================
    # main loop over n chunks
    # =======================================================================
    NPAIR = NMC // 2  # 12 DoubleRow pairs + 1 tail chunk (64 rows)
    for j in range(NNC):
        js = slice(j * NW, (j + 1) * NW)

        f1s = []
        for cc in range(CC):
            fs = sb.tile([P, NW], F32, tag="f1s", bufs=5, name=f"f1s_{j}_{cc}")
            nc.sync.dma_start(fs, fm1[cc * P:(cc + 1) * P, js])
            f1s.append(fs)

        outs = [None] * CC
        for mat, fb, fT, rneg in ((2, f2b, f2T, rneg2), (3, f3b, f3T, rneg3)):
            # scores (transposed, fp8 DoubleRow over channel pairs) + exp
            E = sb.tile([P, NMC, NW], FP8, tag=f"E{mat}", bufs=2,
                        name=f"E{mat}_{j}")
            for mc in range(NMC):
                mw = _mw(mc)
                msl = slice(mc * P, mc * P + mw)
                sp = ps.tile([P, NW], F32, tag="sp", bufs=3,
                             name=f"sp_{j}_{mat}_{mc}")
                for i in range(CC // 2):
                    nc.tensor.matmul(sp[:mw, :],
                                     fb[:, 2 * i:2 * i + 2, msl],
                                     qn[:, 2 * i:2 * i + 2, js],
                                     start=(i == 0), stop=(i == CC // 2 - 1),
                                     perf_mode=DR)
                nc.scalar.activation(E[:mw, mc, :], sp[:mw, :],
                                     mybir.ActivationFunctionType.Exp,
                                     bias=0.0, scale=rneg[:mw, mc:mc + 1])

            # softmax denominator (broadcast over partitions) -> FACTOR/denom
            cs = ps.tile([P, NW], F32, tag="cs", bufs=2, name=f"cs_{j}_{mat}")
            for i in range(NPAIR):
                nc.tensor.matmul(cs, ones128, E[:, 2 * i:2 * i + 2, :],
                                 start=(i == 0), stop=False, perf_mode=DR)
            nc.tensor.matmul(cs, ones128[:MTAIL, 0, :], E[:MTAIL, NMC - 1, :],
                             start=False, stop=True)
            rs = sb.tile([P, NW], F32, tag="rs", bufs=4, name=f"rs_{j}_{mat}")
            nc.vector.reciprocal(rs, cs)
            nc.vector.tensor_scalar_mul(rs, rs, FACTOR)

            # values: nu[c, n] = sum_m fT[m, c] * E[m, n]
            for cc in range(CC):
                csl = slice(cc * P, (cc + 1) * P)
                vp = ps.tile([P, NW], F32, tag="vp", bufs=2,
                             name=f"vp_{j}_{mat}_{cc}")
                for i in range(NPAIR):
                    nc.tensor.matmul(vp, fT[:, 2 * i:2 * i + 2, csl],
                                     E[:, 2 * i:2 * i + 2, :],
                                     start=(i == 0), stop=False, perf_mode=DR)
                nc.tensor.matmul(vp, fT[:MTAIL, NMC - 1, csl],
                                 E[:MTAIL, NMC - 1, :],
                                 start=False, stop=True)
                tmp = sb.tile([P, NW], F32, tag="t", bufs=2,
                              name=f"t_{j}_{mat}_{cc}")
                nc.vector.tensor_mul(tmp, vp, rs)
                if mat == 2:
                    o = sb.tile([P, NW], F32, tag="outs", bufs=5,
                                name=f"o_{j}_{cc}")
                    nc.vector.tensor_add(o, tmp, f1s[cc])
                    outs[cc] = o
                else:
                    o = outs[cc]
                    nc.vector.tensor_add(o, o, tmp)
                    nc.sync.dma_start(out_ap[cc * P:(cc + 1) * P, js], o)


def _build():
    nc = bacc.Bacc("TRN2", target_bir_lowering=False, debug=False,
                   num_devices=B)
    fm1 = nc.dram_tensor("fm1", [C, HW], F32, kind="ExternalInput").ap()
    fm2 = nc.dram_tensor("fm2", [C, HW], F32, kind="ExternalInput").ap()
    fm3 = nc.dram_tensor("fm3", [C, HW], F32, kind="ExternalInput").ap()
    out = nc.dram_tensor("out", [C, HW], F32, kind="ExternalOutput").ap()

    with tile.TileContext(nc) as tc:
        with tc.tile_pool(name="sb", bufs=1) as sb, \
             tc.tile_pool(name="ps", bufs=1, space="PSUM") as ps:
            tc._probe_sb_pool = sb
            tc._probe_ps_pool = ps
            _build_kernel_body(tc, out, fm1, fm2, fm3)
    nc.compile()
    return nc


def _get_nc():
    global _cached_nc
    if _cached_nc is None:
        _cached_nc = _build()
    return _cached_nc


def kernel(**inputs):
    fm1 = np.ascontiguousarray(
        np.asarray(inputs["fm1"], dtype=np.float32).reshape(B, C, HW))
    fm2 = np.ascontiguousarray(
        np.asarray(inputs["fm2"], dtype=np.float32).reshape(B, C, HW))
    fm3 = np.ascontiguousarray(
        np.asarray(inputs["fm3"], dtype=np.float32).reshape(B, C, HW))

    nc = _get_nc()
    in_maps = [{"fm1": fm1[b], "fm2": fm2[b], "fm3": fm3[b]} for b in range(B)]
    res = run_bass_kernel_spmd(nc, in_maps, core_ids=list(range(B)),
                               trace=TRACE)
    kernel.last_results = res
    out = np.stack([res.results[b]["out"] for b in range(B)])
    return out.reshape(B, C, H, W).astype(np.float32)


if __name__ == "__main__":
    rng = np.random.default_rng(0)
    ins = {k: rng.standard_normal((B, C, H, W)).astype(np.float32)
           for k in ("fm1", "fm2", "fm3")}
    o = kernel(**ins)
    print("out shape", o.shape, o.dtype)


# revision 40
# speedup vs baseline: 9.7428x; 2.0382x over previous
"""Trainium2 Bass kernel for nn_FDM_46394236731667.

Computes, per batch b (b = 0..7, one NeuronCore each):
    f1,f2,f3 = fm{1,2,3}[b].reshape(C, HW)
    qn  = f1 / max(||f1||_col, eps)  (column-wise L2 over channels)
    s_k = -(qn^T @ (f_k / max(||f_k||_col, eps)))          k in {2,3}
    a_k = softmax(s_k, axis=-1)
    out[b] = f1 + 0.001 * (f2 @ a2^T + f3 @ a3^T)

Implementation notes:
  - Scores are computed TRANSPOSED (tiles [m_partition, n_free]) so that the
    key-norm scale r_k[m] is a per-partition scalar folded into the ACT exp
    instruction (out = exp(scale[p] * psum)), the softmax denominator is a
    partition-reduction done with an all-ones matmul (result arrives already
    broadcast across partitions), and the P@V matmul consumes the exp tiles
    directly with no transposes of probability tiles.
  - Query norms r1[n] ride on the free axis, so they are pre-folded into the
    query operand qn = 16 * f1 * broadcast(r1) (the 16 makes fp8 quantization
    of unit-norm entries land in the normal range; the matching 1/16 is folded
    into the exp scale).
  - All three big matmul families (scores, softmax-denominator, values) run in
    fp8(e4m3) with perf_mode=DoubleRow: operands are stored with the
    contraction-chunk index as a middle AP dim so each matmul contracts 256
    rows. The output is fm1 + 0.001*(attention terms), so low-precision
    attention arithmetic perturbs the output by ~1e-6 relative.
  - Softmax skips the max-subtraction: scores are cosine similarities in
    [-1, 1], so exp() cannot overflow.
"""
import os
import sys

for _p in ("/opt/trn_rl_repo", "/root/.axon_site/_ro/trn_rl_repo"):
    if os.path.isdir(_p) and _p not in sys.path:
        sys.path.insert(0, _p)

import numpy as np

import concourse.bass as bass
import concourse.tile as tile
from concourse import bacc, mybir
from concourse.bass_utils import run_bass_kernel_spmd
from concourse.masks import make_identity

B, C, H, W = 8, 512, 56, 56
HW = H * W            # 3136
P = 128
CC = C // P           # 4 channel chunks
NMC = 25              # m chunks: 24 x 128 + 1 x 64
MTAIL = HW - 24 * P   # 64
NNC = 7               # n chunks
NW = HW // NNC        # 448
EPS = 1e-12
FACTOR = 0.001
QSCALE = 16.0         # fp8 headroom scale on qn; 1/QSCALE folded into exp

dt = mybir.dt
F32, BF16, FP8 = dt.float32, dt.bfloat16, dt.float8e4
DR = mybir.MatmulPerfMode.DoubleRow

TRACE = False
_cached_nc = None


def _mw(mc):
    return P if mc < NMC - 1 else MTAIL


def _build_preproc(tc, sbP, pre, ps, fm1, fm2, fm3):
    """sbP: persistent operand pool; pre: preproc transients (released before
    the main loop so its SBUF is reused for the main-loop pool)."""
    nc = tc.nc

    # ---- constants ----
    # fp8e5 identity: transposes run on e4m3 data bit-reinterpreted as e5m2
    # (pure data movement; our data never hits the e5m2 Inf/NaN encodings)
    ident = sbP.tile([P, P], dt.float8e3, tag="ident", name="ident")
    make_identity(nc, ident)
    ones128 = sbP.tile([P, 2, P], FP8, tag="ones128", name="ones128")
    nc.vector.memset(ones128, 1.0)
    ones_col = sbP.tile([P, 1], FP8, tag="ones_col", name="ones_col")
    nc.vector.memset(ones_col, 1.0)

    # =======================================================================
    # f1 phase: qn = QSCALE * f1 * broadcast(1/max(||f1||, eps)), fp8,
    # stored as [P, CC, HW] so score matmuls can take [P, 2, NW] slices.
    # =======================================================================
    # f1 squares are pre-scaled by 1/QSCALE so the resulting reciprocal norm
    # comes out as QSCALE/||f1|| with no extra pass.
    fsq1 = []
    fr1b = []
    for cc in range(CC):
        fr = pre.tile([P, HW], F32, tag="fraw", bufs=8, name=f"f1raw_{cc}")
        nc.sync.dma_start(fr, fm1[cc * P:(cc + 1) * P, :])
        t8 = pre.tile([P, HW], FP8, tag="fsq", bufs=4, name=f"fsq1_{cc}")
        nc.scalar.activation(t8, fr, mybir.ActivationFunctionType.Square,
                             bias=0.0, scale=1.0 / QSCALE)
        fsq1.append(t8)
        fr1b.append(fr)

    # rbf[p, n] = QSCALE/max(||f1[:,n]||, eps); qn produced per n-chunk so the
    # first score matmuls can start before the whole f1 phase finishes.
    rbf = pre.tile([P, HW], F32, tag="rbf", name="rbf")
    qn = sbP.tile([P, CC, HW], FP8, tag="qn", name="qn")
    for j in range(NNC):
        js = slice(j * NW, (j + 1) * NW)
        ssb = ps.tile([P, NW], F32, tag="cs", bufs=2, name=f"ss1b_{j}")
        for cc in range(CC):
            nc.tensor.matmul(ssb, ones128[:, 0, :], fsq1[cc][:, js],
                             start=(cc == 0), stop=(cc == CC - 1))
        ns = pre.tile([P, NW], F32, tag="rtmp", bufs=2, name=f"ns1_{j}")
        nc.scalar.sqrt(ns, ssb)
        nc.vector.tensor_scalar_max(ns, ns, EPS / QSCALE)
        nc.vector.reciprocal(rbf[:, js], ns)
        for cc in range(CC):
            nc.vector.tensor_mul(qn[:, cc, js], fr1b[cc][:, js], rbf[:, js])

    # =======================================================================
    # f2/f3 phases: fp8 key operand [P, CC, HW], fp8 transpose [P, NMC, C],
    # per-column key norms (as [P, NMC] columns for the exp scale)
    # =======================================================================
    def key_norms(fm, label):
        fb = sbP.tile([P, CC, HW], FP8, tag=f"{label}b", name=f"{label}b")
        fT = sbP.tile([P, NMC, C], FP8, tag=f"{label}T", name=f"{label}T")
        fsq = []
        for cc in range(CC):
            fr = pre.tile([P, HW], F32, tag="fraw", bufs=8,
                          name=f"{label}raw_{cc}")
            nc.sync.dma_start(fr, fm[cc * P:(cc + 1) * P, :])
            nc.vector.tensor_copy(fb[:, cc, :], fr)
            t8 = pre.tile([P, HW], FP8, tag="fsq", bufs=4,
                          name=f"{label}sq_{cc}")
            nc.scalar.square(t8, fr)
            fsq.append(t8)

        # ss columns: ssc[:mw, mc] = sum_c f[c, mc*128+p]^2
        ssc = ps.tile([P, NMC], F32, tag="cs", bufs=2, name=f"ssc_{label}")
        for mc in range(NMC):
            mw = _mw(mc)
            msl = slice(mc * P, mc * P + mw)
            for cc in range(CC):
                nc.tensor.matmul(ssc[:mw, mc:mc + 1], fsq[cc][:, msl], ones_col,
                                 start=(cc == 0), stop=(cc == CC - 1))
        nrm = pre.tile([P, NMC], F32, tag="rtmp2", bufs=2, name=f"nrm_{label}")
        nc.scalar.sqrt(nrm, ssc)
        nc.vector.tensor_scalar_max(nrm, nrm, EPS)
        rcp = pre.tile([P, NMC], F32, tag="rtmp3", bufs=2, name=f"rcp_{label}")
        nc.vector.reciprocal(rcp, nrm)
        rneg = sbP.tile([P, NMC], F32, tag=f"rneg_{label}", name=f"rneg_{label}")
        nc.vector.tensor_scalar_mul(rneg, rcp, -1.0 / QSCALE)
        return fb, fT, rneg

    E3 = dt.float8e3

    def transpose_one(fT, fb, label, cc, mc):
        # fT[p, mc, c] = f[c, mc*128+p]; PE transpose of the fp8 key operand
        # (bytes viewed as e5m2). The tp psum tiles share the "vp" tag: the
        # first value-matmul psum needs the transposes finished anyway, while
        # the score psum rotation ("sp" tag) stays free of them.
        mw = _mw(mc)
        msl = slice(mc * P, mc * P + mw)
        # fp8 transpose mode writes psum with element step 2
        tp = ps.tile([P, 2 * P], E3, tag="vp", bufs=2,
                     name=f"tp_{label}_{cc}_{mc}")
        tpv = tp[:mw, :].rearrange("p (x two) -> p x two", two=2)[:, :, 0]
        nc.tensor.transpose(tpv, fb[:, cc, msl].bitcast(E3), ident)
        nc.vector.tensor_copy(fT[:mw, mc, cc * P:(cc + 1) * P].bitcast(E3),
                              tpv)

    f2b, f2T, rneg2 = key_norms(fm2, "k2")
    f3b, f3T, rneg3 = key_norms(fm3, "k3")
    # Interleaved k2/k3 transpose work items, dripped into the jp=0 score
    # emission so they fill PE/DVE idle slots under the ACT-paced exp stream.
    tjobs = []
    for cc in range(CC):
        for mc in range(NMC):
            tjobs.append((f2T, f2b, "k2", cc, mc))
            tjobs.append((f3T, f3b, "k3", cc, mc))
    emit = [0]

    def drip_transposes(k):
        hi = min(emit[0] + k, len(tjobs))
        for i in range(emit[0], hi):
            transpose_one(*tjobs[i])
        emit[0] = hi

    return dict(ones128=ones128, qn=qn, drip=drip_transposes,
                mats=((2, f2b, f2T, rneg2), (3, f3b, f3T, rneg3)))


def _build_main(tc, sb, ps, out_ap, fm1, st):
    nc = tc.nc
    ones128 = st["ones128"]
    qn = st["qn"]
    mats = st["mats"]
    drip = st["drip"]

    # =======================================================================
    # main loop over n-chunk pairs: (0,1),(2,3),(4,5),(6,)
    # Scores for both chunks of a pair land in one 2-bank psum tile so a
    # single ACT exp (per-partition scale is identical) covers both.
    # =======================================================================
    NPAIR = NMC // 2  # 12 DoubleRow pairs + 1 tail chunk (64 rows)
    for jp in range((NNC + 1) // 2):
        j0 = 2 * jp
        npj = 2 if j0 + 1 < NNC else 1
        jss = [slice((j0 + jj) * NW, (j0 + jj + 1) * NW) for jj in range(npj)]

        Es = {}
        for mat, fb, fT, rneg in mats:
            E = sb.tile([P, NMC, 2, NW], FP8, tag=f"E{mat}", bufs=2,
                        name=f"E{mat}_{jp}")
            Es[mat] = E
            for mc in range(NMC):
                mw = _mw(mc)
                msl = slice(mc * P, mc * P + mw)
                # [128, 1024] spans 2 psum banks; halves at 0 and 512 so each
                # matmul output stays inside one bank
                sp = ps.tile([P, 1024], F32, tag="sp", bufs=2,
                             name=f"sp_{jp}_{mat}_{mc}")
                for i in range(CC // 2):
                    for jj in range(npj):
                        nc.tensor.matmul(sp[:mw, jj * 512:jj * 512 + NW],
                                         fb[:, 2 * i:2 * i + 2, msl],
                                         qn[:, 2 * i:2 * i + 2, jss[jj]],
                                         start=(i == 0),
                                         stop=(i == CC // 2 - 1),
                                         perf_mode=DR)
                spv = sp[:mw, :].rearrange("p (t x) -> p t x", t=2)
                nc.scalar.activation(E[:mw, mc, :npj, :], spv[:, :npj, :NW],
                                     mybir.ActivationFunctionType.Exp,
                                     bias=0.0, scale=rneg[:mw, mc:mc + 1])
                if jp == 0:
                    drip(4)
        if jp == 0:
            drip(1000)  # flush any remaining transpose jobs

        for mat, fb, fT, rneg in mats:
            E = Es[mat]
            for jj in range(npj):
                js = jss[jj]
                # softmax denominator (broadcast over partitions)
                cs = ps.tile([P, NW], F32, tag="cs", bufs=2,
                             name=f"cs_{jp}_{jj}_{mat}")
                for i in range(NPAIR):
                    nc.tensor.matmul(cs, ones128, E[:, 2 * i:2 * i + 2, jj, :],
                                     start=(i == 0), stop=False, perf_mode=DR)
                nc.tensor.matmul(cs, ones128[:MTAIL, 0, :],
                                 E[:MTAIL, NMC - 1, jj, :],
                                 start=False, stop=True)
                rs = sb.tile([P, NW], F32, tag="rs", bufs=2,
                             name=f"rs_{jp}_{jj}_{mat}")
                nc.vector.reciprocal(rs, cs)
                nc.vector.tensor_scalar_mul(rs, rs, FACTOR)

                # values: nu[c, n] = sum_m fT[m, c] * E[m, n]
                for cc in range(CC):
                    csl = slice(cc * P, (cc + 1) * P)
                    vp = ps.tile([P, NW], F32, tag="vp", bufs=2,
                                 name=f"vp_{jp}_{jj}_{mat}_{cc}")
                    for i in range(NPAIR):
                        nc.tensor.matmul(vp, fT[:, 2 * i:2 * i + 2, csl],
                                         E[:, 2 * i:2 * i + 2, jj, :],
                                         start=(i == 0), stop=False,
                                         perf_mode=DR)
                    nc.tensor.matmul(vp, fT[:MTAIL, NMC - 1, csl],
                                     E[:MTAIL, NMC - 1, jj, :],
                                     start=False, stop=True)
                    tmp = sb.tile([P, NW], F32, tag="t", bufs=3,
                                  name=f"t_{jp}_{jj}_{mat}_{cc}")
                    nc.vector.tensor_mul(tmp, vp, rs)
                    if mat == 2:
                        # out = f1 + tmp2, streamed straight to DRAM
                        fs = sb.tile([P, NW], F32, tag="f1s", bufs=3,
                                     name=f"f1s_{jp}_{jj}_{cc}")
                        nc.sync.dma_start(fs, fm1[cc * P:(cc + 1) * P, js])
                        o = sb.tile([P, NW], F32, tag="outs", bufs=3,
                                    name=f"o_{jp}_{jj}_{cc}")
                        nc.vector.tensor_add(o, tmp, fs)
                        nc.sync.dma_start(out_ap[cc * P:(cc + 1) * P, js], o)
                    else:
                        # accumulate the mat3 contribution in DRAM via DMA
                        nc.gpsimd.dma_start(out_ap[cc * P:(cc + 1) * P, js],
                                            tmp,
                                            accum_op=mybir.AluOpType.add)


def _build():
    nc = bacc.Bacc("TRN2", target_bir_lowering=False, debug=False,
                   num_devices=B)
    fm1 = nc.dram_tensor("fm1", [C, HW], F32, kind="ExternalInput").ap()
    fm2 = nc.dram_tensor("fm2", [C, HW], F32, kind="ExternalInput").ap()
    fm3 = nc.dram_tensor("fm3", [C, HW], F32, kind="ExternalInput").ap()
    out = nc.dram_tensor("out", [C, HW], F32, kind="ExternalOutput").ap()

    with tile.TileContext(nc) as tc:
        with tc.tile_pool(name="sbP", bufs=1) as sbP, \
             tc.tile_pool(name="ps", bufs=1, space="PSUM") as ps:
            with tc.tile_pool(name="pre", bufs=1) as pre:
                st = _build_preproc(tc, sbP, pre, ps, fm1, fm2, fm3)
            with tc.tile_pool(name="sbm", bufs=1) as sbm:
                _build_main(tc, sbm, ps, out, fm1, st)
    nc.compile()
    return nc


def _get_nc():
    global _cached_nc
    if _cached_nc is None:
        _cached_nc = _build()
    return _cached_nc


def kernel(**inputs):
    fm1 = np.ascontiguousarray(
        np.asarray(inputs["fm1"], dtype=np.float32).reshape(B, C, HW))
    fm2 = np.ascontiguousarray(
        np.asarray(inputs["fm2"], dtype=np.float32).reshape(B, C, HW))
    fm3 = np.ascontiguousarray(
        np.asarray(inputs["fm3"], dtype=np.float32).reshape(B, C, HW))

    nc = _get_nc()
    in_maps = [{"fm1": fm1[b], "fm2": fm2[b], "fm3": fm3[b]} for b in range(B)]
    res = run_bass_kernel_spmd(nc, in_maps, core_ids=list(range(B)),
                               trace=TRACE)
    kernel.last_results = res
    out = np.stack([res.results[b]["out"] for b in range(B)])
    return out.reshape(B, C, H, W).astype(np.float32)


if __name__ == "__main__":
    rng = np.random.default_rng(0)
    ins = {k: rng.standard_normal((B, C, H, W)).astype(np.float32)
           for k in ("fm1", "fm2", "fm3")}
    o = kernel(**ins)
    print("out shape", o.shape, o.dtype)
